# revision 2
# baseline (speedup 1.0000x reference)
"""Segment-mean aggregation on 8 trn2 NeuronCores, v3: count-sorted layout.

Host sorts segments by edge count and deals them round-robin to cores, so
each core's tile t holds 128 segments of near-identical count. Each segment
in a tile is padded to the tile max count m_t (padding gathers a zero table
row). Slotting: chunk j of tile t holds edge j of each of the 128 segments
(slot p = segment p of the tile).

Device per tile t:
  - m_t matmuls: psum[seg, feat] += I.T @ gath_chunk[slot, feat]
    (lhsT = constant fp16 identity; rhs = raw gathered chunk, N=64) --
    the PE is a PSUM accumulator over the per-segment edge chunks
  - ScalarE copies PSUM [128,64] to the resident output buffer with
    per-partition scale = 1/count (the segment mean)
Gathers are one 128-row indirect DMA per chunk (HW gathers exactly one row
per partition per call). One 8 MB output DMA at the end. Host does the
final rank->segment unpermute.
"""

import sys

import numpy as np

sys.path.insert(0, "/opt/trn_rl_repo")

from concourse import bacc, bass, mybir
import concourse.tile as tile
from concourse.bass_utils import run_bass_kernel_spmd

N_SRC = 1_000_000
E = 4_000_000
S = 250_000
D = 64

N_CORES = 8
P = 128
S_CORE = S // N_CORES            # 31250 segments per core
TILES = (S_CORE + P - 1) // P    # 245 tiles (last partial: 31360 slots)

LAST_EXEC_NS = None
LAST_RESULTS = None


def _host_prep(values, gather_idx, segment_ids):
    vals = np.asarray(values, dtype=np.float32)
    idx = np.asarray(gather_idx).astype(np.int64, copy=False)
    seg = np.asarray(segment_ids).astype(np.int64, copy=False)

    table = np.zeros((N_SRC + 1, D), dtype=np.float16)
    table[:N_SRC] = vals.astype(np.float16)   # row N_SRC = padding zeros

    counts = np.bincount(seg, minlength=S)               # [S]
    order = np.argsort(counts, kind="stable")            # rank -> segment id
    rank_of = np.empty(S, dtype=np.int64)                # segment id -> rank
    rank_of[order] = np.arange(S)
    counts_sorted = counts[order]

    # tile t spans ranks [1024*t, 1024*(t+1)); m_t = max count in block
    m = np.zeros(TILES, dtype=np.int64)
    for t in range(TILES):
        hi = min(1024 * (t + 1), S)
        m[t] = max(int(counts_sorted[1024 * t:hi].max(initial=0)), 1)
    cum = np.concatenate([[0], np.cumsum(m)])
    tot_chunks = int(cum[-1])

    # per-edge slot: segment s -> rank r; core k=r%8, pos=r//8,
    # tile t=pos//128, row p=pos%128, col=cum[t] + j (j = index within seg)
    r_e = rank_of[seg]                                   # [E]
    k_e = r_e % N_CORES
    pos_e = r_e // N_CORES
    t_e = pos_e // P
    p_e = pos_e % P
    starts = np.concatenate([[0], np.cumsum(counts)])    # per segment id
    j_e = np.arange(E) - starts[seg]
    col_e = cum[t_e] + j_e

    offs = np.full((N_CORES, P, tot_chunks), N_SRC, dtype=np.int32)
    offs[k_e, p_e, col_e] = idx.astype(np.int32)

    # per (core, row, tile) reciprocal count
    ranks = (np.arange(N_CORES)[:, None] +
             N_CORES * np.arange(S_CORE)[None, :])       # [8, 31250]
    cnt_k = counts_sorted[ranks]                         # [8, 31250]
    rec_k = (1.0 / np.maximum(cnt_k, 1)).astype(np.float32)
    rec_full = np.zeros((N_CORES, TILES * P), dtype=np.float32)
    rec_full[:, :S_CORE] = rec_k
    rec = np.ascontiguousarray(
        rec_full.reshape(N_CORES, TILES, P).transpose(0, 2, 1))

    eye = np.eye(P, dtype=np.float16)

    return table, offs, rec, eye, m, cum, tot_chunks, order


def _build_program(m, cum, tot_chunks, repeats=1):
    dt = mybir.dt
    nc = bacc.Bacc()
    table_d = nc.declare_dram_parameter(
        "table", [N_SRC + 1, D], dt.float16, isOutput=False)
    offs_d = nc.declare_dram_parameter(
        "offs", [P, tot_chunks], dt.int32, isOutput=False)
    rec_d = nc.declare_dram_parameter(
        "rec", [P, TILES], dt.float32, isOutput=False)
    eye_d = nc.declare_dram_parameter(
        "eye", [P, P], dt.float16, isOutput=False)
    out_d = nc.declare_dram_parameter(
        "outp", [P, TILES * D], dt.float32, isOutput=True)

    with tile.TileContext(nc) as tc:
        with (
            tc.tile_pool(name="const", bufs=1) as cpool,
            tc.tile_pool(name="gath", bufs=8) as gpool,
            tc.tile_pool(name="psum", bufs=6, space="PSUM") as ppool,
        ):
            eye_sb = cpool.tile([P, P], dt.float16)
            nc.sync.dma_start(out=eye_sb[:], in_=eye_d[:])
            offs_sb = cpool.tile([P, tot_chunks], dt.int32)
            nc.sync.dma_start(out=offs_sb[:], in_=offs_d[:])
            rec_sb = cpool.tile([P, TILES], dt.float32)
            nc.sync.dma_start(out=rec_sb[:], in_=rec_d[:])
            out_sb = cpool.tile([P, TILES * D], dt.float32)

            for _rep in range(repeats):
                for t in range(TILES):
                    n_t = int(m[t])
                    base = int(cum[t])
                    ps = ppool.tile([P, D], dt.float32)
                    for c in range(n_t):
                        cc = base + c
                        gt = gpool.tile([P, D], dt.float16, tag="gath")
                        nc.gpsimd.indirect_dma_start(
                            out=gt[:],
                            out_offset=None,
                            in_=table_d[:],
                            in_offset=bass.IndirectOffsetOnAxis(
                                ap=offs_sb[:, cc:cc + 1], axis=0),
                        )
                        nc.tensor.matmul(
                            out=ps[:],
                            lhsT=eye_sb[:],
                            rhs=gt[:],
                            start=(c == 0),
                            stop=(c == n_t - 1),
                        )
                    nc.scalar.activation(
                        out=out_sb[:, t * D:(t + 1) * D], in_=ps[:],
                        func=mybir.ActivationFunctionType.Copy,
                        scale=rec_sb[:, t:t + 1])
                nc.sync.dma_start(out=out_d[:], in_=out_sb[:])
    nc.finalize()
    return nc


def _decode(results, order):
    """results[k]["outp"] [128, TILES*64] -> full [S, 64] in original ids."""
    out = np.empty((S, D), dtype=np.float32)
    pos = np.arange(S_CORE)
    for k in range(N_CORES):
        o = results[k]["outp"].reshape(P, TILES, D).transpose(1, 0, 2)
        o = o.reshape(TILES * P, D)[:S_CORE]        # row = pos = rank//8
        segids = order[k + N_CORES * pos]
        out[segids] = o
    return out


def kernel(values, gather_idx, segment_ids, num_segments, trace=False):
    global LAST_EXEC_NS, LAST_RESULTS
    table, offs, rec, eye, m, cum, tot_chunks, order = _host_prep(
        values, gather_idx, segment_ids)

    nc = _build_program(m, cum, tot_chunks)

    in_maps = [
        {"table": table, "offs": offs[k], "rec": rec[k], "eye": eye}
        for k in range(N_CORES)
    ]
    res = run_bass_kernel_spmd(
        nc, in_maps, list(range(N_CORES)), trace=trace)
    LAST_EXEC_NS = res.exec_time_ns
    LAST_RESULTS = res

    return _decode(res.results, order)
